# revision 2
# baseline (speedup 1.0000x reference)
"""Trainium2 Bass kernel for nn_ColorRestoration.

Math (per image row, W = 3072, w_ceil = 14, RGB_IDX = (3, 7, 10)):
    u_c[t]   = x[t + idx_c] * z[t]                (x zero-padded on the right)
    y[c, p]  = ms14(u_c)[p] / ms14(z)[p]          (backward moving sums, width 14)
    rgb[c,p] = z[p - idx_c]                       (z zero-padded on the left)

All ops are per-row along W, so H (2048 rows) shards across the 8 cores with
zero communication: 256 rows per core.

Per-core device kernel: rows sit on SBUF partitions (2 row-tiles of 128),
W is processed in column chunks.  Each width-14 moving sum is ONE DVE
tensor_tensor_scan:  state = (u[t] + state) - u[t-14], chained across chunks
via the scan's `initial` operand.  The u products run on GPSIMD in parallel
with the scans; the reciprocal of the z moving sum and the final normalize
multiplies run on DVE.  rgb is a pure shifted copy of the z tile, DMA'd
straight out of SBUF.

The kernel is HBM-bound (24 MiB of DMA per core vs ~20 us of compute per
engine), so the layout optimizes DMA efficiency: large column chunks
(~0.8 MB per transfer), loads + y0/y1 stores issued on the SP HWDGE ring,
rgb + y2 stores on the ACT HWDGE ring, and the normalized y is written back
into the (dead) u-product buffer to keep SBUF small enough for double
buffering.
"""

import sys

sys.path.insert(0, "/opt/trn_rl_repo")

import numpy as np

import concourse.bass as bass
import concourse.mybir as mybir
import concourse.tile as tile
from concourse import bass_utils

F32 = mybir.dt.float32
OP = mybir.AluOpType
G = 14  # w_ceil: moving-sum width == left guard columns
XG = 13  # right guard for x (max shift is idx_c <= 13)
RGB_IDX = (3, 7, 10)
N_CORES = 8
H, W = 2048, 3072
HS = H // N_CORES  # rows per core


def split_waits(nc, maxw=1):
    """Split multi-wait instructions into single-wait NOPs.

    The walrus codegen in this container rejects instructions carrying more
    than a couple of sync waits ("Too many sync wait commands").  Waiting on
    [w1..wN] then executing I equals NOP(w1); ...; I(wN) on the same engine,
    since each engine executes its block subsequence in order.
    """
    uid = 0
    for f in nc.m.functions:
        for b in f.blocks:
            out, changed = [], False
            for ins in b.instructions:
                si = ins.sync_info
                if si is not None and len(si.on_wait) > maxw:
                    waits = list(si.on_wait)
                    keep, rest = waits[-maxw:], waits[:-maxw]
                    for i in range(0, len(rest), maxw):
                        nop = mybir.InstNoOp(
                            name=f"splitw-{uid}", engine=ins.engine
                        )
                        uid += 1
                        nop.sync_info = mybir.SyncInfo(
                            on_wait=rest[i : i + maxw], on_update=[]
                        )
                        nc.register_instruction(nop)
                        out.append(nop)
                    ins.sync_info = mybir.SyncInfo(
                        on_wait=keep, on_update=list(si.on_update)
                    )
                    changed = True
                out.append(ins)
            if changed:
                b.instructions = out


def build_nc(hs=HS, w=W, cw=1536, bufs=2, reps=1):
    """Build the per-core Bass program: x,z [hs,w] -> y,rgb [3,hs,w]."""
    assert hs % 128 == 0 and w % cw == 0 and cw >= G
    nc = bass.Bass("TRN2", debug=False)
    x = nc.dram_tensor("x", [hs, w], F32, kind="ExternalInput")
    z = nc.dram_tensor("z", [hs, w], F32, kind="ExternalInput")
    y = nc.dram_tensor("y", [3, hs, w], F32, kind="ExternalOutput")
    rgb = nc.dram_tensor("rgb", [3, hs, w], F32, kind="ExternalOutput")

    with tile.TileContext(nc) as tc:
        with tc.tile_pool(name="pool", bufs=bufs) as pool:
          for _rep in range(reps):
            for rt in range(hs // 128):
                r0 = rt * 128
                rows = slice(r0, r0 + 128)
                carry = [0.0, 0.0, 0.0]
                carry_z = 0.0
                for j in range(w // cw):
                    cs, ce = j * cw, (j + 1) * cw
                    # x_buf covers x[rows, cs-G : ce+XG], z_buf z[rows, cs-G : ce]
                    x_buf = pool.tile([128, G + cw + XG], F32, tag="x")
                    z_buf = pool.tile([128, G + cw], F32, tag="z")
                    xl, xr = cs - G, ce + XG
                    vlo, vhi = max(xl, 0), min(xr, w)
                    if vlo > xl:
                        nc.gpsimd.memset(x_buf[:, : vlo - xl], 0.0)
                        nc.gpsimd.memset(z_buf[:, : vlo - xl], 0.0)
                    if xr > vhi:
                        nc.gpsimd.memset(x_buf[:, vhi - xl :], 0.0)
                    nc.sync.dma_start(x_buf[:, vlo - xl : vhi - xl], x[rows, vlo:vhi])
                    nc.sync.dma_start(z_buf[:, vlo - xl :], z[rows, vlo:ce])

                    # rgb[c][p] = z[p - idx_c]: shifted view of z_buf
                    for c, idx in enumerate(RGB_IDX):
                        nc.scalar.dma_start(
                            rgb[c, rows, cs:ce], z_buf[:, G - idx : G - idx + cw]
                        )

                    # denominator: ms14(z) in one scan, then reciprocal
                    msz = pool.tile([128, cw], F32, tag="msz")
                    nc.vector.tensor_tensor_scan(
                        msz[:, :], z_buf[:, G : G + cw], z_buf[:, 0:cw],
                        carry_z, op0=OP.add, op1=OP.subtract,
                    )
                    carry_z = msz[:, cw - 1 : cw]
                    rcp = pool.tile([128, cw], F32, tag="rcp")
                    nc.vector.reciprocal(rcp[:, :], msz[:, :])

                    for c, idx in enumerate(RGB_IDX):
                        u = pool.tile([128, G + cw], F32, tag=f"u{c}")
                        nc.gpsimd.tensor_tensor(
                            u[:, :], x_buf[:, idx : idx + G + cw],
                            z_buf[:, :], op=OP.mult,
                        )
                        ms = pool.tile([128, cw], F32, tag=f"ms{c}")
                        nc.vector.tensor_tensor_scan(
                            ms[:, :], u[:, G : G + cw], u[:, 0:cw],
                            carry[c], op0=OP.add, op1=OP.subtract,
                        )
                        carry[c] = ms[:, cw - 1 : cw]
                        # y = ms * rcp, written into the dead part of u
                        nc.vector.tensor_tensor(
                            u[:, G : G + cw], ms[:, :], rcp[:, :], op=OP.mult
                        )
                        eng = nc.sync if c < 2 else nc.scalar
                        eng.dma_start(y[c, rows, cs:ce], u[:, G : G + cw])

    split_waits(nc, maxw=1)
    return nc


_NC_CACHE = {}


def _get_nc(hs, w, cw):
    key = (hs, w, cw)
    if key not in _NC_CACHE:
        _NC_CACHE[key] = build_nc(hs, w, cw)
    return _NC_CACHE[key]


def run_sharded(x2, z2, cw=1536, trace=False, **kw):
    """x2, z2: [H, W] float32.  Returns (y, rgb) [3, H, W] (+ results obj)."""
    h, w = x2.shape
    hs = h // N_CORES
    nc = _get_nc(hs, w, cw)
    in_maps = [
        {
            "x": np.ascontiguousarray(x2[i * hs : (i + 1) * hs]),
            "z": np.ascontiguousarray(z2[i * hs : (i + 1) * hs]),
        }
        for i in range(N_CORES)
    ]
    res = bass_utils.run_bass_kernel_spmd(
        nc, in_maps, list(range(N_CORES)), trace=trace, **kw
    )
    yf = np.concatenate([res.results[i]["y"] for i in range(N_CORES)], axis=1)
    rf = np.concatenate([res.results[i]["rgb"] for i in range(N_CORES)], axis=1)
    return yf, rf, res


def kernel(x, z):
    x2 = np.asarray(x, dtype=np.float32).reshape(H, W)
    z2 = np.asarray(z, dtype=np.float32).reshape(H, W)
    yf, rf, _ = run_sharded(x2, z2)
    return yf.reshape(1, 3, H, W), rf.reshape(1, 3, H, W)


# revision 5
# speedup vs baseline: 1.5716x; 1.5716x over previous
"""Trainium2 Bass kernel for nn_ColorRestoration.

Math (per image row, W = 3072, w_ceil = 14, RGB_IDX = (3, 7, 10)):
    u_c[t]   = x[t + idx_c] * z[t]                (x zero-padded on the right)
    y[c, p]  = ms14(u_c)[p] / ms14(z)[p]          (backward moving sums, width 14)
    rgb[c,p] = z[p - idx_c]                       (z zero-padded on the left)

All ops are per-row along W, so H (2048 rows) shards across the 8 cores with
zero communication: 256 rows per core.

Per-core device kernel: rows sit on SBUF partitions (2 row-tiles of 128),
W is processed full-width (cw=3072).  Each width-14 moving sum is ONE DVE
tensor_tensor_scan:  state = (u[t] + state) - u[t-14].  rgb is a pure
shifted copy of the z tile, DMA'd straight out of SBUF.

Engine budget per row-tile (measured rates: DVE scan 8.9us, DVE TT 4.1us,
DVE reciprocal 23.7us(!), GPSIMD TT 8.0us, DMA ~370 GB/s/direction):
  DVE    : 4 scans + 3 normalize mults (~48us/rt) -- the critical engine
  GPSIMD : 3 u-products only (feeds scans; anything downstream of a scan
           on the in-order GPSIMD queue head-of-line-blocks the next
           row-tile's products and doubles total time -- measured)
  ACT    : reciprocal as exp(-ln(msz)) (the Reciprocal/Rsqrt activation
           is banned for accuracy; ln+exp is fine at rel-tol 2e-2 and
           DVE reciprocal costs 23.7us) + issues half the DMAs
  SP     : issues the other half of the DMAs
DMA byte split: sync ring = x, z loads + y0, y2 stores; scalar ring =
rgb0-2 + y1 stores (~6.3 MB each per row-tile).
"""

import sys

sys.path.insert(0, "/opt/trn_rl_repo")

import numpy as np

import concourse.bass as bass
import concourse.mybir as mybir
import concourse.tile as tile
from concourse import bass_utils

F32 = mybir.dt.float32
OP = mybir.AluOpType
AF = mybir.ActivationFunctionType
G = 14  # w_ceil: moving-sum width == left guard columns
XG = 13  # right guard for x (max shift is idx_c <= 13)
RGB_IDX = (3, 7, 10)
N_CORES = 8
H, W = 2048, 3072
HS = H // N_CORES  # rows per core


def split_waits(nc, maxw=1):
    """Split multi-wait instructions into single-wait NOPs.

    The walrus codegen in this container rejects instructions carrying more
    than a couple of sync waits ("Too many sync wait commands").  Waiting on
    [w1..wN] then executing I equals NOP(w1); ...; I(wN) on the same engine,
    since each engine executes its block subsequence in order.
    """
    uid = 0
    for f in nc.m.functions:
        for b in f.blocks:
            out, changed = [], False
            for ins in b.instructions:
                si = ins.sync_info
                if si is not None and len(si.on_wait) > maxw:
                    waits = list(si.on_wait)
                    keep, rest = waits[-maxw:], waits[:-maxw]
                    for i in range(0, len(rest), maxw):
                        nop = mybir.InstNoOp(
                            name=f"splitw-{uid}", engine=ins.engine
                        )
                        uid += 1
                        nop.sync_info = mybir.SyncInfo(
                            on_wait=rest[i : i + maxw], on_update=[]
                        )
                        nc.register_instruction(nop)
                        out.append(nop)
                    ins.sync_info = mybir.SyncInfo(
                        on_wait=keep, on_update=list(si.on_update)
                    )
                    changed = True
                out.append(ins)
            if changed:
                b.instructions = out


def build_nc(hs=HS, w=W, cw=3072, bufs=2, reps=1):
    """Build the per-core Bass program: x,z [hs,w] -> y,rgb [3,hs,w]."""
    assert hs % 128 == 0 and w % cw == 0 and cw >= G
    nc = bass.Bass("TRN2", debug=False)
    x = nc.dram_tensor("x", [hs, w], F32, kind="ExternalInput")
    z = nc.dram_tensor("z", [hs, w], F32, kind="ExternalInput")
    y = nc.dram_tensor("y", [3, hs, w], F32, kind="ExternalOutput")
    rgb = nc.dram_tensor("rgb", [3, hs, w], F32, kind="ExternalOutput")
    # carry APs cross chunk boundaries: a scan that overwrites a
    # single-buffered ms/msz tile would also read its last column ->
    # scheduler deadlock.  Double-buffer them when chunks > 1.
    mb = 2 if w // cw > 1 else 1

    with tile.TileContext(nc) as tc:
        with tc.tile_pool(name="pool", bufs=1) as pool:
          for _rep in range(reps):
            for rt in range(hs // 128):
                r0 = rt * 128
                rows = slice(r0, r0 + 128)
                carry = [0.0, 0.0, 0.0]
                carry_z = 0.0
                for j in range(w // cw):
                    cs, ce = j * cw, (j + 1) * cw
                    # x_buf covers x[rows, cs-G : ce+XG], z_buf z[rows, cs-G : ce]
                    x_buf = pool.tile([128, G + cw + XG], F32, tag="x", bufs=bufs)
                    z_buf = pool.tile([128, G + cw], F32, tag="z", bufs=bufs)
                    xl, xr = cs - G, ce + XG
                    vlo, vhi = max(xl, 0), min(xr, w)
                    if vlo > xl:
                        nc.vector.memset(x_buf[:, : vlo - xl], 0.0)
                        nc.vector.memset(z_buf[:, : vlo - xl], 0.0)
                    if xr > vhi:
                        nc.vector.memset(x_buf[:, vhi - xl :], 0.0)
                    nc.sync.dma_start(x_buf[:, vlo - xl : vhi - xl], x[rows, vlo:vhi])
                    nc.sync.dma_start(z_buf[:, vlo - xl :], z[rows, vlo:ce])

                    # rgb[c][p] = z[p - idx_c]: shifted view of z_buf
                    for c, idx in enumerate(RGB_IDX):
                        nc.scalar.dma_start(
                            rgb[c, rows, cs:ce], z_buf[:, G - idx : G - idx + cw]
                        )

                    # denominator: ms14(z) in one scan; 1/msz on ACT
                    msz = pool.tile([128, cw], F32, tag="msz", bufs=mb)
                    nc.vector.tensor_tensor_scan(
                        msz[:, :], z_buf[:, G : G + cw], z_buf[:, 0:cw],
                        carry_z, op0=OP.add, op1=OP.subtract,
                    )
                    carry_z = msz[:, cw - 1 : cw]
                    lnt = pool.tile([128, cw], F32, tag="lnt")
                    rcp = pool.tile([128, cw], F32, tag="rcp")
                    nc.scalar.activation(lnt[:, :], msz[:, :], AF.Ln)
                    nc.scalar.activation(rcp[:, :], lnt[:, :], AF.Exp, scale=-1.0)

                    for c, idx in enumerate(RGB_IDX):
                        u = pool.tile([128, G + cw], F32, tag=f"u{c}", bufs=bufs)
                        nc.gpsimd.tensor_tensor(
                            u[:, :], x_buf[:, idx : idx + G + cw],
                            z_buf[:, :], op=OP.mult,
                        )
                        ms = pool.tile([128, cw], F32, tag=f"ms{c}", bufs=mb)
                        nc.vector.tensor_tensor_scan(
                            ms[:, :], u[:, G : G + cw], u[:, 0:cw],
                            carry[c], op0=OP.add, op1=OP.subtract,
                        )
                        carry[c] = ms[:, cw - 1 : cw]
                        # y = ms * rcp, written into the dead part of u
                        nc.vector.tensor_tensor(
                            u[:, G : G + cw], ms[:, :], rcp[:, :], op=OP.mult
                        )
                        eng = nc.scalar if c == 1 else nc.sync
                        eng.dma_start(y[c, rows, cs:ce], u[:, G : G + cw])

    split_waits(nc, maxw=1)
    return nc


_NC_CACHE = {}


def _get_nc(hs, w, cw):
    key = (hs, w, cw)
    if key not in _NC_CACHE:
        _NC_CACHE[key] = build_nc(hs, w, cw)
    return _NC_CACHE[key]


def run_sharded(x2, z2, cw=3072, trace=False, **kw):
    """x2, z2: [H, W] float32.  Returns (y, rgb) [3, H, W] (+ results obj)."""
    h, w = x2.shape
    hs = h // N_CORES
    nc = _get_nc(hs, w, cw)
    in_maps = [
        {
            "x": np.ascontiguousarray(x2[i * hs : (i + 1) * hs]),
            "z": np.ascontiguousarray(z2[i * hs : (i + 1) * hs]),
        }
        for i in range(N_CORES)
    ]
    res = bass_utils.run_bass_kernel_spmd(
        nc, in_maps, list(range(N_CORES)), trace=trace, **kw
    )
    yf = np.concatenate([res.results[i]["y"] for i in range(N_CORES)], axis=1)
    rf = np.concatenate([res.results[i]["rgb"] for i in range(N_CORES)], axis=1)
    return yf, rf, res


def kernel(x, z):
    x2 = np.asarray(x, dtype=np.float32).reshape(H, W)
    z2 = np.asarray(z, dtype=np.float32).reshape(H, W)
    yf, rf, _ = run_sharded(x2, z2)
    return yf.reshape(1, 3, H, W), rf.reshape(1, 3, H, W)
